# revision 12
# baseline (speedup 1.0000x reference)
# Adaptive softmax (head 2002 + tail0 8000 + tail1 40000 -> [4096, 50000] log-probs)
# on 8 TRN2 NeuronCores, data-parallel over the 4096 tokens (512 tokens/core).
#
# Per core: all matmuls run in bf16 on the TensorEngine (weights pre-transposed
# and cast on host), log-softmax statistics accumulated with the ScalarEngine's
# Exp+accum_out on wide tiles, and the corrected f32 rows DMA'd straight to the
# output. head/tail0 logits are materialized in SBUF (fp16) so their weights
# stream once; tail1 (the 40000-wide projection) is computed twice (sumexp pass
# + output pass) to avoid materializing 20MB of logits, trading PE flops for
# HBM traffic. Weight streams are deep-prefetched (4 bufs) so the PE never
# waits on HBM.
import os
import sys

for _p in (
    "/root/.axon_site",
    "/root/.axon_site/_ro/trn_rl_repo",
    "/root/.axon_site/_ro/pypackages",
    "/opt/trn_rl_repo",
    "/opt/pypackages",
):
    if os.path.isdir(_p) and _p not in sys.path:
        sys.path.append(_p)

import ml_dtypes
import numpy as np

import concourse.bass as bass
import concourse.mybir as mybir
import concourse.tile as tile
from concourse import bacc
from concourse.bass_utils import run_bass_kernel_spmd

B = 4096  # tokens total
D = 1024  # hidden
NCORES = 8
T = B // NCORES  # 512 tokens per core
MCH = T // 128  # 4 token chunks of 128
KD = D // 128  # 8 k-tiles for D
OUT_HEAD = 2002
C0 = 2000
V0 = 8000  # tail0 vocab width
V1 = 40000  # tail1 vocab width
H1 = 256  # tail1 reduced hidden
K1 = H1 // 128  # 2
C2 = 50000
T0_OFF = 2000  # output column offset of tail0 block
T1_OFF = 10000  # output column offset of tail1 block

BF16 = mybir.dt.bfloat16
FP8 = mybir.dt.float8e4  # TRN e4m3 (max +-240)
W1B_SCALE = 16.0  # host pre-scale on W1b fp8 copy
H1_SCALE = 4.0  # device pre-scale on h1 fp8 copy
F16 = mybir.dt.float16
F32 = mybir.dt.float32
AF = mybir.ActivationFunctionType
ALU = mybir.AluOpType
X_AXIS = mybir.AxisListType.X


def _blocks(width, bw):
    return [(o, min(bw, width - o)) for o in range(0, width, bw)]


def _r(ap):
    # DRAM [K, N] viewed as [p, a, n] so one DMA loads all K-tiles of a column block
    return ap.rearrange("(a p) n -> p a n", p=128)


def build():
    nc = bacc.Bacc(None, target_bir_lowering=False)
    xT = nc.declare_dram_parameter("xT", [D, T], BF16, isOutput=False)
    WhT = nc.declare_dram_parameter("WhT", [D, OUT_HEAD], BF16, isOutput=False)
    bh = nc.declare_dram_parameter("bh", [1, OUT_HEAD], BF16, isOutput=False)
    W0aT = nc.declare_dram_parameter("W0aT", [D, D], BF16, isOutput=False)
    W0bT = nc.declare_dram_parameter("W0bT", [D, V0], BF16, isOutput=False)
    W1aT = nc.declare_dram_parameter("W1aT", [D, H1], BF16, isOutput=False)
    W1bT = nc.declare_dram_parameter("W1bT", [H1, V1], BF16, isOutput=False)
    W1bT8 = nc.declare_dram_parameter("W1bT8", [H1, V1], FP8, isOutput=False)
    out = nc.declare_dram_parameter("out", [T, C2], F32, isOutput=True)

    t1_blocks = _blocks(V1, 2048)  # 20 streamed blocks for tail1
    n_t1b = len(t1_blocks)

    with tile.TileContext(nc) as tc:
        with (
            tc.tile_pool(name="const", bufs=1) as cpool,
            tc.tile_pool(name="logits", bufs=1) as lpool,
            tc.tile_pool(name="stats", bufs=1) as spool,
            tc.tile_pool(name="wblk", bufs=4) as wpool,
            tc.tile_pool(name="wblk1", bufs=2) as w1pool,
            tc.tile_pool(name="wblk8", bufs=3) as w8pool,
            tc.tile_pool(name="scr", bufs=2) as scpool,
            tc.tile_pool(name="stage", bufs=2) as stpool,
        ):
            # ---- resident inputs -------------------------------------------------
            xT_sb = cpool.tile([128, KD, T], BF16)
            nc.sync.dma_start(out=xT_sb[:], in_=_r(xT[:]))
            w0a_sb = cpool.tile([128, KD, D], BF16)
            nc.sync.dma_start(out=w0a_sb[:], in_=_r(W0aT[:]))
            w1a_sb = cpool.tile([128, KD, H1], BF16)
            nc.sync.dma_start(out=w1a_sb[:], in_=_r(W1aT[:]))
            bh_sb = cpool.tile([1, OUT_HEAD], BF16)
            nc.sync.dma_start(out=bh_sb[:], in_=bh[:])
            ones_sb = cpool.tile([1, 128], BF16)
            nc.vector.memset(ones_sb[:], 1.0)

            h0T = cpool.tile([128, KD, T], BF16)  # (x @ W0a.T).T, hid on partitions
            h1T = cpool.tile([128, K1, T], BF16)  # (x @ W1a.T).T
            h1T8 = cpool.tile([128, K1, T], FP8)  # h1 * 4, fp8 copy for pass 1

            # ---- stats -----------------------------------------------------------
            t0_sums = spool.tile([128, MCH, 4], F32)
            t1_sums = spool.tile([128, MCH, n_t1b], F32)
            se_head = spool.tile([128, MCH], F32)
            se_t0 = spool.tile([128, MCH], F32)
            se_t1 = spool.tile([128, MCH], F32)
            lse_head = spool.tile([128, MCH], F32)
            lse_t0 = spool.tile([128, MCH], F32)
            lse_t1 = spool.tile([128, MCH], F32)
            c01 = spool.tile([128, MCH, 2], F32)  # head cluster logits (f32)
            neg_head = spool.tile([128, MCH], F32)
            tmp0 = spool.tile([128, MCH], F32)
            tmp1 = spool.tile([128, MCH], F32)
            neg0 = spool.tile([128, MCH], F32)
            neg1 = spool.tile([128, MCH], F32)

            head_logits = lpool.tile([128, MCH, OUT_HEAD], F16)
            t0_logits = lpool.tile([128, MCH, V0], F16)

            # ============== phases with 512-wide psum groups ======================
            with tc.tile_pool(
                name="psA", bufs=6, space=bass.MemorySpace.PSUM
            ) as ppoolA:

                def psum512():
                    return ppoolA.tile([128, 512], F32, tag="ps", name="ps")

                # ---- phase H: hidden projections h0T / h1T -----------------------
                for dst, wsb, nchunk in ((h0T, w0a_sb, KD), (h1T, w1a_sb, K1)):
                    for hc in range(nchunk):
                        ps = psum512()
                        for k in range(KD):
                            nc.tensor.matmul(
                                ps[:],
                                wsb[:, k, hc * 128 : (hc + 1) * 128],
                                xT_sb[:, k, :],
                                start=(k == 0),
                                stop=(k == KD - 1),
                            )
                        nc.vector.tensor_copy(dst[:, hc, :], ps[:])
                        if dst is h1T:
                            nc.scalar.mul(h1T8[:, hc, :], ps[:], H1_SCALE)

                # ---- HEAD + TAIL0: stream weights once, materialize fp16 logits --
                def mm_section(wdram, width, lhsT_sb, kt, visit, with_bias):
                    """stream [128, kt, 512] blocks; per (block, m) one psum group"""
                    for bo, bw in _blocks(width, 512):
                        wb = wpool.tile([128, KD, 512], BF16, tag="wblk")
                        nc.sync.dma_start(
                            out=wb[:, :kt, :bw], in_=_r(wdram[:])[:, :, bo : bo + bw]
                        )
                        for m in range(MCH):
                            ms = slice(m * 128, (m + 1) * 128)
                            ps = psum512()
                            for k in range(kt):
                                nc.tensor.matmul(
                                    ps[:, :bw],
                                    lhsT_sb[:, k, ms],
                                    wb[:, k, :bw],
                                    start=(k == 0),
                                    stop=(k == kt - 1 and not with_bias),
                                )
                            if with_bias:
                                nc.tensor.matmul(
                                    ps[:, :bw],
                                    ones_sb[:, :],
                                    bh_sb[:, bo : bo + bw],
                                    start=False,
                                    stop=True,
                                )
                            visit(m, bo, bw, ps)

                def head_visit(m, go, vw, ps):
                    nc.vector.tensor_copy(head_logits[:, m, go : go + vw], ps[:, :vw])
                    if go + vw == OUT_HEAD:
                        nc.vector.tensor_copy(c01[:, m, :], ps[:, vw - 2 : vw])

                def t0_visit(m, go, vw, ps):
                    nc.vector.tensor_copy(t0_logits[:, m, go : go + vw], ps[:, :vw])

                mm_section(WhT, OUT_HEAD, xT_sb, KD, head_visit, True)
                mm_section(W0bT, V0, h0T, KD, t0_visit, False)

                # head/t0 softmax stats from the materialized logits (wide ACT ops)
                for m in range(MCH):
                    sc = scpool.tile([128, 2048], F16, tag="expsc")
                    nc.scalar.activation(
                        sc[:, :OUT_HEAD],
                        head_logits[:, m, :],
                        AF.Exp,
                        accum_out=se_head[:, m : m + 1],
                    )
                    for j, (so, sw) in enumerate(_blocks(V0, 2048)):
                        sc = scpool.tile([128, 2048], F16, tag="expsc")
                        nc.scalar.activation(
                            sc[:, :sw],
                            t0_logits[:, m, so : so + sw],
                            AF.Exp,
                            accum_out=t0_sums[:, m, j : j + 1],
                        )
                    nc.vector.tensor_reduce(
                        se_t0[:, m : m + 1], t0_sums[:, m, :], X_AXIS, ALU.add
                    )
                # batched Ln (single ACT table residency), then per-m biases
                nc.scalar.activation(lse_head[:, :], se_head[:, :], AF.Ln)
                nc.scalar.activation(lse_t0[:, :], se_t0[:, :], AF.Ln)
                nc.vector.tensor_scalar_mul(neg_head[:, :], lse_head[:, :], -1.0)
                # bias = c0 - lse_head - lse_t0
                nc.vector.tensor_sub(tmp0[:, :], c01[:, :, 0], lse_head[:, :])
                nc.vector.tensor_sub(neg0[:, :], tmp0[:, :], lse_t0[:, :])

                # head output rows: out[:, :2000] = head_logits - lse_head (on DVE,
                # ACT is the critical engine through this span)
                for m in range(MCH):
                    ms = slice(m * 128, (m + 1) * 128)
                    st = stpool.tile([128, 2048], F32, tag="stage")
                    nc.vector.tensor_scalar_add(
                        st[:, :C0], head_logits[:, m, :C0], neg_head[:, m : m + 1]
                    )
                    nc.gpsimd.dma_start(out=out[ms, 0:C0], in_=st[:, :C0])

                # t0 output: out[:, 2000:10000] = t0_logits + (c0 - lse_head - lse_t0)
                for m in range(MCH):
                    ms = slice(m * 128, (m + 1) * 128)
                    for so, sw in _blocks(V0, 2048):
                        st = stpool.tile([128, 2048], F32, tag="stage")
                        nc.vector.tensor_scalar_add(
                            st[:, :sw],
                            t0_logits[:, m, so : so + sw],
                            neg0[:, m : m + 1],
                        )
                        nc.gpsimd.dma_start(
                            out=out[ms, T0_OFF + so : T0_OFF + so + sw], in_=st[:, :sw]
                        )

            # ============== tail1 phases with 2048-wide psum groups ===============
            with tc.tile_pool(
                name="psB", bufs=2, space=bass.MemorySpace.PSUM
            ) as ppoolB:

                def psum2k():
                    return ppoolB.tile([128, 2048], F32, tag="ps2", name="ps2")

                def t1_group(m, wb, bw, ps):
                    ms = slice(m * 128, (m + 1) * 128)
                    for vo, vw in _blocks(bw, 512):
                        for k in range(K1):
                            nc.tensor.matmul(
                                ps[:, vo : vo + vw],
                                h1T[:, k, ms],
                                wb[:, k, vo : vo + vw],
                                start=(k == 0),
                                stop=(k == K1 - 1),
                            )

                # ---- TAIL1 pass 1: sumexp only (exp in place on psum) ------------
                for bi, (bo, bw) in enumerate(t1_blocks):
                    wb8 = w8pool.tile([128, K1, 2048], FP8, tag="wblk8")
                    nc.sync.dma_start(
                        out=wb8[:, :, :bw], in_=_r(W1bT8[:])[:, :, bo : bo + bw]
                    )
                    for m in range(MCH):
                        ms = slice(m * 128, (m + 1) * 128)
                        ps = psum2k()
                        for vo, vw in _blocks(bw, 512):
                            # fp8 DoubleRow: one matmul contracts both K-tiles
                            nc.tensor.matmul(
                                ps[:, vo : vo + vw],
                                h1T8[:, :, ms],
                                wb8[:, :, vo : vo + vw],
                                perf_mode=mybir.MatmulPerfMode.DoubleRow,
                                start=True,
                                stop=True,
                            )
                        nc.scalar.activation(
                            ps[:, :bw],
                            ps[:, :bw],
                            AF.Exp,
                            scale=1.0 / (W1B_SCALE * H1_SCALE),
                            accum_out=t1_sums[:, m, bi : bi + 1],
                        )

                for m in range(MCH):
                    nc.vector.tensor_reduce(
                        se_t1[:, m : m + 1], t1_sums[:, m, :], X_AXIS, ALU.add
                    )
                nc.scalar.activation(lse_t1[:, :], se_t1[:, :], AF.Ln)
                # bias = c1 - lse_head - lse_t1
                nc.vector.tensor_sub(tmp1[:, :], c01[:, :, 1], lse_head[:, :])
                nc.vector.tensor_sub(neg1[:, :], tmp1[:, :], lse_t1[:, :])

                # ---- TAIL1 pass 2: recompute logits, correct, write --------------
                for bo, bw in t1_blocks:
                    wb = w1pool.tile([128, K1, 2048], BF16, tag="wblk1")
                    nc.sync.dma_start(
                        out=wb[:, :, :bw], in_=_r(W1bT[:])[:, :, bo : bo + bw]
                    )
                    for m in range(MCH):
                        ms = slice(m * 128, (m + 1) * 128)
                        ps = psum2k()
                        t1_group(m, wb, bw, ps)
                        st = stpool.tile([128, 2048], F32, tag="stage")
                        # split the psum drain across ACT and DVE so neither
                        # engine gates the PE's next accumulation group
                        half = (bw // 2 + 3) & ~3
                        nc.scalar.activation(
                            st[:, :half],
                            ps[:, :half],
                            AF.Identity,
                            bias=neg1[:, m : m + 1],
                        )
                        nc.vector.tensor_scalar_add(
                            st[:, half:bw], ps[:, half:bw], neg1[:, m : m + 1]
                        )
                        nc.gpsimd.dma_start(
                            out=out[ms, T1_OFF + bo : T1_OFF + bo + bw], in_=st[:, :bw]
                        )

    nc.compile()
    return nc


_NC_CACHE = {}


def _get_nc():
    if "nc" not in _NC_CACHE:
        _NC_CACHE["nc"] = build()
    return _NC_CACHE["nc"]


def _prep_weights(Wh, bh, W0a, W0b, W1a, W1b):
    f = ml_dtypes.bfloat16
    return {
        "WhT": np.ascontiguousarray(np.asarray(Wh, np.float32).T).astype(f),
        "bh": np.asarray(bh, np.float32).reshape(1, OUT_HEAD).astype(f),
        "W0aT": np.ascontiguousarray(np.asarray(W0a, np.float32).T).astype(f),
        "W0bT": np.ascontiguousarray(np.asarray(W0b, np.float32).T).astype(f),
        "W1aT": np.ascontiguousarray(np.asarray(W1a, np.float32).T).astype(f),
        "W1bT": np.ascontiguousarray(np.asarray(W1b, np.float32).T).astype(f),
        "W1bT8": (np.ascontiguousarray(np.asarray(W1b, np.float32).T) * W1B_SCALE
                  ).astype(ml_dtypes.float8_e4m3),
    }


def kernel(x, Wh, bh, W0a, W0b, W1a, W1b, _trace=False):
    x = np.asarray(x, np.float32)
    nc = _get_nc()
    shared = _prep_weights(Wh, bh, W0a, W0b, W1a, W1b)
    in_maps = []
    for i in range(NCORES):
        m = dict(shared)
        m["xT"] = np.ascontiguousarray(x[i * T : (i + 1) * T].T).astype(
            ml_dtypes.bfloat16
        )
        in_maps.append(m)
    res = run_bass_kernel_spmd(nc, in_maps, core_ids=list(range(NCORES)), trace=_trace)
    out = np.concatenate([res.results[i]["out"] for i in range(NCORES)], axis=0)
    if _trace:
        return out, res
    return out


# revision 14
# speedup vs baseline: 1.1209x; 1.1209x over previous
# Adaptive softmax (head 2002 + tail0 8000 + tail1 40000 -> [4096, 50000] log-probs)
# on 8 TRN2 NeuronCores, data-parallel over the 4096 tokens (512 tokens/core).
#
# Per core: all matmuls run in bf16 on the TensorEngine (weights pre-transposed
# and cast on host), log-softmax statistics accumulated with the ScalarEngine's
# Exp+accum_out on wide tiles, and the corrected f32 rows DMA'd straight to the
# output. head/tail0 logits are materialized in SBUF (fp16) so their weights
# stream once; tail1 (the 40000-wide projection) is computed twice (sumexp pass
# + output pass) to avoid materializing 20MB of logits, trading PE flops for
# HBM traffic. Weight streams are deep-prefetched (4 bufs) so the PE never
# waits on HBM.
import os
import sys

for _p in (
    "/root/.axon_site",
    "/root/.axon_site/_ro/trn_rl_repo",
    "/root/.axon_site/_ro/pypackages",
    "/opt/trn_rl_repo",
    "/opt/pypackages",
):
    if os.path.isdir(_p) and _p not in sys.path:
        sys.path.append(_p)

import ml_dtypes
import numpy as np

import concourse.bass as bass
import concourse.mybir as mybir
import concourse.tile as tile
from concourse import bacc
from concourse.bass_utils import run_bass_kernel_spmd

B = 4096  # tokens total
D = 1024  # hidden
NCORES = 8
T = B // NCORES  # 512 tokens per core
MCH = T // 128  # 4 token chunks of 128
KD = D // 128  # 8 k-tiles for D
OUT_HEAD = 2002
C0 = 2000
V0 = 8000  # tail0 vocab width
V1 = 40000  # tail1 vocab width
H1 = 256  # tail1 reduced hidden
K1 = H1 // 128  # 2
C2 = 50000
T0_OFF = 2000  # output column offset of tail0 block
T1_OFF = 10000  # output column offset of tail1 block

BF16 = mybir.dt.bfloat16
FP8 = mybir.dt.float8e4  # TRN e4m3 (max +-240)
W1B_SCALE = 16.0  # host pre-scale on W1b fp8 copy
H1_SCALE = 4.0  # device pre-scale on h1 fp8 copy
F16 = mybir.dt.float16
F32 = mybir.dt.float32
AF = mybir.ActivationFunctionType
ALU = mybir.AluOpType
X_AXIS = mybir.AxisListType.X


def _blocks(width, bw):
    return [(o, min(bw, width - o)) for o in range(0, width, bw)]


def _r(ap):
    # DRAM [K, N] viewed as [p, a, n] so one DMA loads all K-tiles of a column block
    return ap.rearrange("(a p) n -> p a n", p=128)


def build():
    nc = bacc.Bacc(None, target_bir_lowering=False)
    xT = nc.declare_dram_parameter("xT", [D, T], BF16, isOutput=False)
    WhT = nc.declare_dram_parameter("WhT", [D, OUT_HEAD], BF16, isOutput=False)
    bh = nc.declare_dram_parameter("bh", [1, OUT_HEAD], BF16, isOutput=False)
    W0aT = nc.declare_dram_parameter("W0aT", [D, D], BF16, isOutput=False)
    W0bT = nc.declare_dram_parameter("W0bT", [D, V0], BF16, isOutput=False)
    W1aT = nc.declare_dram_parameter("W1aT", [D, H1], BF16, isOutput=False)
    W1bT = nc.declare_dram_parameter("W1bT", [H1, V1], BF16, isOutput=False)
    W1bT8 = nc.declare_dram_parameter("W1bT8", [H1, V1], FP8, isOutput=False)
    out = nc.declare_dram_parameter("out", [T, C2], F32, isOutput=True)

    t1_blocks = _blocks(V1, 2048)  # 20 streamed blocks for tail1
    n_t1b = len(t1_blocks)

    with tile.TileContext(nc) as tc:
        with (
            tc.tile_pool(name="const", bufs=1) as cpool,
            tc.tile_pool(name="logits", bufs=1) as lpool,
            tc.tile_pool(name="stats", bufs=1) as spool,
            tc.tile_pool(name="wblk", bufs=3) as wpool,
            tc.tile_pool(name="wblk1", bufs=3) as w1pool,
            tc.tile_pool(name="wblk8", bufs=2) as w8pool,
            tc.tile_pool(name="scr", bufs=1) as scpool,
            tc.tile_pool(name="stage", bufs=3) as stpool,
        ):
            # ---- resident inputs -------------------------------------------------
            xT_sb = cpool.tile([128, KD, T], BF16)
            nc.sync.dma_start(out=xT_sb[:], in_=_r(xT[:]))
            w0a_sb = cpool.tile([128, KD, D], BF16)
            nc.sync.dma_start(out=w0a_sb[:], in_=_r(W0aT[:]))
            w1a_sb = cpool.tile([128, KD, H1], BF16)
            nc.sync.dma_start(out=w1a_sb[:], in_=_r(W1aT[:]))
            bh_sb = cpool.tile([1, OUT_HEAD], BF16)
            nc.sync.dma_start(out=bh_sb[:], in_=bh[:])
            ones_sb = cpool.tile([1, 128], BF16)
            nc.vector.memset(ones_sb[:], 1.0)

            h0T = cpool.tile([128, KD, T], BF16)  # (x @ W0a.T).T, hid on partitions
            h1T = cpool.tile([128, K1, T], BF16)  # (x @ W1a.T).T
            h1T8 = cpool.tile([128, K1, T], FP8)  # h1 * 4, fp8 copy for pass 1

            # ---- stats -----------------------------------------------------------
            t0_sums = spool.tile([128, MCH, 4], F32)
            t1_sums = spool.tile([128, MCH, n_t1b], F32)
            se_head = spool.tile([128, MCH], F32)
            se_t0 = spool.tile([128, MCH], F32)
            se_t1 = spool.tile([128, MCH], F32)
            lse_head = spool.tile([128, MCH], F32)
            lse_t0 = spool.tile([128, MCH], F32)
            lse_t1 = spool.tile([128, MCH], F32)
            c01 = spool.tile([128, MCH, 2], F32)  # head cluster logits (f32)
            neg_head = spool.tile([128, MCH], F32)
            tmp0 = spool.tile([128, MCH], F32)
            tmp1 = spool.tile([128, MCH], F32)
            neg0 = spool.tile([128, MCH], F32)
            neg1 = spool.tile([128, MCH], F32)

            head_logits = lpool.tile([128, MCH, OUT_HEAD], F16)
            t0_logits = lpool.tile([128, MCH, V0], F16)

            # ============== phases with 512-wide psum groups ======================
            with tc.tile_pool(
                name="psA", bufs=6, space=bass.MemorySpace.PSUM
            ) as ppoolA:

                def psum512():
                    return ppoolA.tile([128, 512], F32, tag="ps", name="ps")

                # ---- phase H: hidden projections h0T / h1T -----------------------
                for dst, wsb, nchunk in ((h0T, w0a_sb, KD), (h1T, w1a_sb, K1)):
                    for hc in range(nchunk):
                        ps = psum512()
                        for k in range(KD):
                            nc.tensor.matmul(
                                ps[:],
                                wsb[:, k, hc * 128 : (hc + 1) * 128],
                                xT_sb[:, k, :],
                                start=(k == 0),
                                stop=(k == KD - 1),
                            )
                        nc.vector.tensor_copy(dst[:, hc, :], ps[:])
                        if dst is h1T:
                            nc.scalar.mul(h1T8[:, hc, :], ps[:], H1_SCALE)

                # ---- HEAD + TAIL0: stream weights once, materialize fp16 logits --
                def mm_section(wdram, width, lhsT_sb, kt, visit, with_bias):
                    """stream [128, kt, 512] blocks; per (block, m) one psum group"""
                    for bo, bw in _blocks(width, 512):
                        wb = wpool.tile([128, KD, 512], BF16, tag="wblk")
                        nc.sync.dma_start(
                            out=wb[:, :kt, :bw], in_=_r(wdram[:])[:, :, bo : bo + bw]
                        )
                        for m in range(MCH):
                            ms = slice(m * 128, (m + 1) * 128)
                            ps = psum512()
                            for k in range(kt):
                                nc.tensor.matmul(
                                    ps[:, :bw],
                                    lhsT_sb[:, k, ms],
                                    wb[:, k, :bw],
                                    start=(k == 0),
                                    stop=(k == kt - 1 and not with_bias),
                                )
                            if with_bias:
                                nc.tensor.matmul(
                                    ps[:, :bw],
                                    ones_sb[:, :],
                                    bh_sb[:, bo : bo + bw],
                                    start=False,
                                    stop=True,
                                )
                            visit(m, bo, bw, ps)

                def head_visit(m, go, vw, ps):
                    nc.vector.tensor_copy(head_logits[:, m, go : go + vw], ps[:, :vw])
                    if go + vw == OUT_HEAD:
                        nc.vector.tensor_copy(c01[:, m, :], ps[:, vw - 2 : vw])

                def t0_visit(m, go, vw, ps):
                    nc.vector.tensor_copy(t0_logits[:, m, go : go + vw], ps[:, :vw])

                mm_section(WhT, OUT_HEAD, xT_sb, KD, head_visit, True)
                mm_section(W0bT, V0, h0T, KD, t0_visit, False)

                # head/t0 softmax stats from the materialized logits (wide ACT ops)
                for m in range(MCH):
                    sc = scpool.tile([128, 2048], F16, tag="expsc")
                    nc.scalar.activation(
                        sc[:, :OUT_HEAD],
                        head_logits[:, m, :],
                        AF.Exp,
                        accum_out=se_head[:, m : m + 1],
                    )
                    for j, (so, sw) in enumerate(_blocks(V0, 2048)):
                        sc = scpool.tile([128, 2048], F16, tag="expsc")
                        nc.scalar.activation(
                            sc[:, :sw],
                            t0_logits[:, m, so : so + sw],
                            AF.Exp,
                            accum_out=t0_sums[:, m, j : j + 1],
                        )
                    nc.vector.tensor_reduce(
                        se_t0[:, m : m + 1], t0_sums[:, m, :], X_AXIS, ALU.add
                    )
                # batched Ln (single ACT table residency), then per-m biases
                nc.scalar.activation(lse_head[:, :], se_head[:, :], AF.Ln)
                nc.scalar.activation(lse_t0[:, :], se_t0[:, :], AF.Ln)
                nc.vector.tensor_scalar_mul(neg_head[:, :], lse_head[:, :], -1.0)
                # bias = c0 - lse_head - lse_t0
                nc.vector.tensor_sub(tmp0[:, :], c01[:, :, 0], lse_head[:, :])
                nc.vector.tensor_sub(neg0[:, :], tmp0[:, :], lse_t0[:, :])

                # head output rows: out[:, :2000] = head_logits - lse_head (on DVE,
                # ACT is the critical engine through this span)
                for m in range(MCH):
                    ms = slice(m * 128, (m + 1) * 128)
                    st = stpool.tile([128, 2048], F32, tag="stage")
                    nc.vector.tensor_scalar_add(
                        st[:, :C0], head_logits[:, m, :C0], neg_head[:, m : m + 1]
                    )
                    nc.gpsimd.dma_start(out=out[ms, 0:C0], in_=st[:, :C0])

                # t0 output: out[:, 2000:10000] = t0_logits + (c0 - lse_head - lse_t0)
                for m in range(MCH):
                    ms = slice(m * 128, (m + 1) * 128)
                    for so, sw in _blocks(V0, 2048):
                        st = stpool.tile([128, 2048], F32, tag="stage")
                        nc.vector.tensor_scalar_add(
                            st[:, :sw],
                            t0_logits[:, m, so : so + sw],
                            neg0[:, m : m + 1],
                        )
                        nc.gpsimd.dma_start(
                            out=out[ms, T0_OFF + so : T0_OFF + so + sw], in_=st[:, :sw]
                        )

            # ============== tail1 phases with 2048-wide psum groups ===============
            with tc.tile_pool(
                name="psB", bufs=2, space=bass.MemorySpace.PSUM
            ) as ppoolB:

                def psum2k():
                    return ppoolB.tile([128, 2048], F32, tag="ps2", name="ps2")

                def t1_group(m, wb, bw, ps):
                    ms = slice(m * 128, (m + 1) * 128)
                    for vo, vw in _blocks(bw, 512):
                        for k in range(K1):
                            nc.tensor.matmul(
                                ps[:, vo : vo + vw],
                                h1T[:, k, ms],
                                wb[:, k, vo : vo + vw],
                                start=(k == 0),
                                stop=(k == K1 - 1),
                            )

                # ---- TAIL1 pass 1: sumexp only (exp in place on psum) ------------
                for bi, (bo, bw) in enumerate(t1_blocks):
                    wb8 = w8pool.tile([128, K1, 2048], FP8, tag="wblk8")
                    nc.sync.dma_start(
                        out=wb8[:, :, :bw], in_=_r(W1bT8[:])[:, :, bo : bo + bw]
                    )
                    for m in range(MCH):
                        ms = slice(m * 128, (m + 1) * 128)
                        ps = psum2k()
                        for vo, vw in _blocks(bw, 512):
                            # fp8 DoubleRow: one matmul contracts both K-tiles
                            nc.tensor.matmul(
                                ps[:, vo : vo + vw],
                                h1T8[:, :, ms],
                                wb8[:, :, vo : vo + vw],
                                perf_mode=mybir.MatmulPerfMode.DoubleRow,
                                start=True,
                                stop=True,
                            )
                        sc = scpool.tile([128, 2048], F16, tag="expsc")
                        nc.scalar.activation(
                            sc[:, :bw],
                            ps[:, :bw],
                            AF.Exp,
                            scale=1.0 / (W1B_SCALE * H1_SCALE),
                            accum_out=t1_sums[:, m, bi : bi + 1],
                        )

                for m in range(MCH):
                    nc.vector.tensor_reduce(
                        se_t1[:, m : m + 1], t1_sums[:, m, :], X_AXIS, ALU.add
                    )
                nc.scalar.activation(lse_t1[:, :], se_t1[:, :], AF.Ln)
                # bias = c1 - lse_head - lse_t1
                nc.vector.tensor_sub(tmp1[:, :], c01[:, :, 1], lse_head[:, :])
                nc.vector.tensor_sub(neg1[:, :], tmp1[:, :], lse_t1[:, :])

                # ---- TAIL1 pass 2: recompute logits, correct, write --------------
                for bo, bw in t1_blocks:
                    wb = w1pool.tile([128, K1, 2048], BF16, tag="wblk1")
                    nc.sync.dma_start(
                        out=wb[:, :, :bw], in_=_r(W1bT[:])[:, :, bo : bo + bw]
                    )
                    for m in range(MCH):
                        ms = slice(m * 128, (m + 1) * 128)
                        ps = psum2k()
                        t1_group(m, wb, bw, ps)
                        st = stpool.tile([128, 2048], F32, tag="stage")
                        # split the psum drain across ACT and DVE so neither
                        # engine gates the PE's next accumulation group
                        half = (bw // 2 + 3) & ~3
                        nc.scalar.activation(
                            st[:, :half],
                            ps[:, :half],
                            AF.Identity,
                            bias=neg1[:, m : m + 1],
                        )
                        nc.vector.tensor_scalar_add(
                            st[:, half:bw], ps[:, half:bw], neg1[:, m : m + 1]
                        )
                        nc.gpsimd.dma_start(
                            out=out[ms, T1_OFF + bo : T1_OFF + bo + bw], in_=st[:, :bw]
                        )

    nc.compile()
    return nc


_NC_CACHE = {}


def _get_nc():
    if "nc" not in _NC_CACHE:
        _NC_CACHE["nc"] = build()
    return _NC_CACHE["nc"]


def _prep_weights(Wh, bh, W0a, W0b, W1a, W1b):
    f = ml_dtypes.bfloat16
    return {
        "WhT": np.ascontiguousarray(np.asarray(Wh, np.float32).T).astype(f),
        "bh": np.asarray(bh, np.float32).reshape(1, OUT_HEAD).astype(f),
        "W0aT": np.ascontiguousarray(np.asarray(W0a, np.float32).T).astype(f),
        "W0bT": np.ascontiguousarray(np.asarray(W0b, np.float32).T).astype(f),
        "W1aT": np.ascontiguousarray(np.asarray(W1a, np.float32).T).astype(f),
        "W1bT": np.ascontiguousarray(np.asarray(W1b, np.float32).T).astype(f),
        "W1bT8": (np.ascontiguousarray(np.asarray(W1b, np.float32).T) * W1B_SCALE
                  ).astype(ml_dtypes.float8_e4m3),
    }


def kernel(x, Wh, bh, W0a, W0b, W1a, W1b, _trace=False):
    x = np.asarray(x, np.float32)
    nc = _get_nc()
    shared = _prep_weights(Wh, bh, W0a, W0b, W1a, W1b)
    in_maps = []
    for i in range(NCORES):
        m = dict(shared)
        m["xT"] = np.ascontiguousarray(x[i * T : (i + 1) * T].T).astype(
            ml_dtypes.bfloat16
        )
        in_maps.append(m)
    res = run_bass_kernel_spmd(nc, in_maps, core_ids=list(range(NCORES)), trace=_trace)
    out = np.concatenate([res.results[i]["out"] for i in range(NCORES)], axis=0)
    if _trace:
        return out, res
    return out
